# revision 13
# baseline (speedup 1.0000x reference)
"""Trainium2 Bass kernel for relative-position multi-head attention.

Problem: B=8, N=1024, DIM=512, H=8, DH=64, MAX_POS=512
  out = softmax(q k^T * s + pos) v @ Wo + bo,  pos[i,r] = q_i . E[clip(i-r)+512] * s

Sharding: data-parallel over batch, one batch element per NeuronCore (8 cores).

Per-core algorithm (transposed layouts, bf16 matmuls, f32 PSUM accum):
  qT/kT  = (Wq*s)^T x^T, Wk^T x^T          (inner, N)
  va     = [x Wv | ones]                   (N, 65 per head) - PV lhsT + Z row
  Gxr[i,u] = q_i . E[clip(639-u)+512]      (reversed q.E table, per head)
     -> DRAM with row pitch 1281 so each banded 128-chunk of pos^T is a
        256B-aligned row: row(i, r_b) at idx = 10*i + 1 + i//128
  dma_gather(transpose=True): g[rr, q, i] = pos^T[r, i] for r_b = i_b-4+q
  S^T(r_b) psum = k_b^T q  (+ identity-matmul accumulate of g slices for the
  banded blocks, + ones x Gsat rank-1 terms for the saturated ranges)
  exp on ScalarE (PSUM -> SBUF bf16); O^T accumulated with ones-augmented V
  (row 64 = softmax denominator Z); normalize by 1/Z; out^T = Wo^T O^T + bo.
  Host transposes back.
"""

import numpy as np
import ml_dtypes
import sys

sys.path.insert(0, "/opt/trn_rl_repo")

import concourse.bass as bass  # noqa: E402
import concourse.mybir as mybir  # noqa: E402
import concourse.tile as tile  # noqa: E402
from concourse import bacc  # noqa: E402
from concourse.bass_utils import run_bass_kernel_spmd  # noqa: E402

B, N, DIM = 8, 1024, 512
H, DH = 8, 64
MAX_POS = 512
SCALE = DH ** -0.5
NB = N // 128          # 8 seq blocks
WGX = 1281             # padded Etxr width: [pad | 1279 cols | pad]
PITCH = 1281           # Gxr DRAM row pitch (elements); payload cols [1, 1280)
NROWS = 10240          # gather-view rows of 128 elems (idx max 10238)
ESIZE = 9 * 128        # gather row: 9 blocks of 128

bf16 = ml_dtypes.bfloat16
BF = mybir.dt.bfloat16
F32 = mybir.dt.float32
I16 = mybir.dt.int16

# full padded width is written per block so the uniform 9-block gather never
# touches uninitialized DRAM (garbage lands only in unused q-slices)


def _ap(base, rel_off, pattern):
    """Custom access pattern relative to a tile's base AP."""
    b = base[:]
    return bass.AP(tensor=b.tensor, offset=b.offset + rel_off, ap=pattern)


def build_bass():
    nc = bacc.Bacc()

    xT = nc.declare_dram_parameter("xT", [DIM, N], BF, isOutput=False)
    wq = nc.declare_dram_parameter("wq", [DIM, DIM], BF, isOutput=False)
    wk = nc.declare_dram_parameter("wk", [DIM, DIM], BF, isOutput=False)
    wv = nc.declare_dram_parameter("wv", [DIM, DIM], BF, isOutput=False)
    wo = nc.declare_dram_parameter("wo", [DIM, DIM], BF, isOutput=False)
    bo = nc.declare_dram_parameter("bo", [128, 4], F32, isOutput=False)
    etxr = nc.declare_dram_parameter("etxr", [128, WGX], BF, isOutput=False)   # rows 64-127 dup
    et2 = nc.declare_dram_parameter("et2", [128, 2], BF, isOutput=False)       # rows 64-127 dup
    iden = nc.declare_dram_parameter("iden", [128, 128], BF, isOutput=False)
    idxs = nc.declare_dram_parameter("idxs", [128, N // 16], I16, isOutput=False)
    out = nc.declare_dram_parameter("out", [DIM, N], F32, isOutput=True)

    with tile.TileContext(nc) as tc, tc.tile_pool(name="consts", bufs=1) as consts, \
            tc.tile_pool(name="qk", bufs=1) as qkpool, \
            tc.tile_pool(name="dram", bufs=2, space="DRAM") as drampool:

        # ---------- load constants ----------
        xT_sb = [consts.tile([128, N], BF, tag=f"xt{i}", name=f"xt{i}") for i in range(4)]
        wq_sb = [consts.tile([128, DIM], BF, tag=f"wq{i}", name=f"wq{i}") for i in range(4)]
        wk_sb = [consts.tile([128, DIM], BF, tag=f"wk{i}", name=f"wk{i}") for i in range(4)]
        wv_sb = [consts.tile([128, DIM], BF, tag=f"wv{i}", name=f"wv{i}") for i in range(4)]
        wo_sb = [consts.tile([128, DIM], BF, tag=f"wo{i}", name=f"wo{i}") for i in range(4)]
        for k in range(4):
            sl = slice(128 * k, 128 * k + 128)
            nc.sync.dma_start(out=xT_sb[k], in_=xT[sl, :])
            nc.sync.dma_start(out=wq_sb[k], in_=wq[sl, :])
            nc.sync.dma_start(out=wk_sb[k], in_=wk[sl, :])
            nc.sync.dma_start(out=wv_sb[k], in_=wv[sl, :])
            nc.sync.dma_start(out=wo_sb[k], in_=wo[sl, :])
        etxr_sb = consts.tile([128, WGX], BF)
        nc.sync.dma_start(out=etxr_sb, in_=etxr[:, :])
        et2_sb = consts.tile([128, 2], BF)
        nc.sync.dma_start(out=et2_sb, in_=et2[:, :])
        iden_sb = consts.tile([128, 128], BF)
        nc.sync.dma_start(out=iden_sb, in_=iden[:, :])
        idxs_sb = consts.tile([128, N // 16], I16)
        nc.sync.dma_start(out=idxs_sb, in_=idxs[:, :])
        bo_sb = consts.tile([128, 4], F32)
        nc.sync.dma_start(out=bo_sb, in_=bo[:, :])
        ones_sb = consts.tile([1, 128], BF)
        nc.vector.memset(ones_sb, 1.0)

        # long-lived activations
        qT_sb = [qkpool.tile([128, N], BF, tag=f"qt{i}", name=f"qt{i}") for i in range(4)]
        kT_sb = [qkpool.tile([128, N], BF, tag=f"kt{i}", name=f"kt{i}") for i in range(4)]
        va_sb = [qkpool.tile([128, H * 65], BF, tag=f"va{i}", name=f"va{i}") for i in range(NB)]
        oT_sb = [qkpool.tile([128, N], BF, tag=f"ot{i}", name=f"ot{i}") for i in range(4)]

        # ---------- projections ----------
        with tc.tile_pool(name="proj_psum", bufs=2, space="PSUM") as pp:
            for m in range(4):
                for c in range(2):
                    csl = slice(512 * c, 512 * c + 512)
                    pq = pp.tile([128, 512], F32, tag="pq")
                    pk = pp.tile([128, 512], F32, tag="pk")
                    for k in range(4):
                        msl = slice(128 * m, 128 * m + 128)
                        nc.tensor.matmul(pq, wq_sb[k][:, msl], xT_sb[k][:, csl],
                                         start=(k == 0), stop=(k == 3))
                        nc.tensor.matmul(pk, wk_sb[k][:, msl], xT_sb[k][:, csl],
                                         start=(k == 0), stop=(k == 3))
                    nc.scalar.copy(out=qT_sb[m][:, csl], in_=pq)
                    nc.scalar.copy(out=kT_sb[m][:, csl], in_=pk)
            for nt in range(NB):
                pv = pp.tile([128, 512], F32, tag="pv")
                for k in range(4):
                    nsl = slice(128 * nt, 128 * nt + 128)
                    nc.tensor.matmul(pv, xT_sb[k][:, nsl], wv_sb[k],
                                     start=(k == 0), stop=(k == 3))
                # scatter v columns into [65*h, 65*h+64) of va; set ones col
                vout = _ap(va_sb[nt], 0, [[H * 65, 128], [65, H], [1, 64]])
                vin = _ap(pv, 0, [[512, 128], [64, H], [1, 64]])
                nc.vector.tensor_copy(vout, vin)
                oc = _ap(va_sb[nt], 64, [[H * 65, 128], [65, H], [1, 1]])
                nc.vector.memset(oc, 1.0)

        # ---------- attention ----------
        with tc.tile_pool(name="gx_psum", bufs=2, space="PSUM") as gxp, \
                tc.tile_pool(name="st_psum", bufs=3, space="PSUM") as stp, \
                tc.tile_pool(name="o_psum", bufs=1, space="PSUM") as op, \
                tc.tile_pool(name="gs_psum", bufs=1, space="PSUM") as gsp, \
                tc.tile_pool(name="gxstage", bufs=3) as gxs, \
                tc.tile_pool(name="gpool", bufs=2) as gpool, \
                tc.tile_pool(name="es", bufs=3) as esp, \
                tc.tile_pool(name="small", bufs=4) as small:

            for h in range(H):
                pair, off = h // 2, 64 * (h % 2)
                hsl = slice(off, off + 64)
                qTh = qT_sb[pair]
                kTh = kT_sb[pair]

                # --- Gxr table -> DRAM ---
                gxr = drampool.tile([N * PITCH], BF)
                for ib in range(NB):
                    stg = gxs.tile([128, WGX], BF, tag="stg")
                    isl = slice(128 * ib, 128 * ib + 128)
                    c0 = 0
                    while c0 < WGX:
                        cw = min(512, WGX - c0)
                        pg = gxp.tile([128, 512], F32, tag="pg")
                        nc.tensor.matmul(pg[:, :cw], qTh[hsl, isl],
                                         etxr_sb[hsl, c0:c0 + cw],
                                         start=True, stop=True)
                        if (c0 // 512) % 2 == 0:
                            nc.scalar.copy(out=stg[:, c0:c0 + cw], in_=pg[:, :cw])
                        else:
                            nc.vector.tensor_copy(stg[:, c0:c0 + cw], pg[:, :cw])
                        c0 += cw
                    dst = _ap(gxr, 128 * ib * PITCH,
                              [[PITCH, 128], [1, WGX]])
                    nc.sync.dma_start(out=dst, in_=stg)

                # --- Gsat rows: q . E[0] (row "lo"), q . E[1024] (row "hi") ---
                gsat = [small.tile([1, N], BF, tag=f"gsat{r}", name=f"gsat{r}") for r in range(2)]
                for r in range(2):
                    for c in range(2):
                        csl = slice(512 * c, 512 * c + 512)
                        ps = gsp.tile([1, 512], F32, tag="ps")
                        nc.tensor.matmul(ps, et2_sb[hsl, r:r + 1], qTh[hsl, csl],
                                         start=True, stop=True)
                        nc.vector.tensor_copy(gsat[r][:, csl], ps)

                # --- gather pos^T for the whole head ---
                g = gpool.tile([128, 9, N], BF, tag="g")
                src = _ap(gxr, 0, [[128, NROWS], [1, ESIZE]])
                nc.gpsimd.dma_gather(
                    out_ap=g[:], in_ap=src, idxs_ap=idxs_sb[:],
                    num_idxs=N, num_idxs_reg=N, elem_size=ESIZE, elem_step=128,
                    transpose=True, single_packet=False,
                )

                # --- per key-block attention ---
                oacc = None
                for rb in range(NB):
                    rsl = slice(128 * rb, 128 * rb + 128)
                    ib_lo, ib_hi = max(0, rb - 4), min(NB, rb + 5)  # banded blocks
                    lo_end = 128 * max(0, rb - 4)    # sat-low: i < lo_end
                    hi_st = 128 * min(NB, rb + 5)    # sat-high: i >= hi_st
                    if rb == 0:
                        oacc = [op.tile([65, 512], F32, tag=f"oacc{c}", name=f"oacc{c}")
                                for c in range(2)]
                    es = esp.tile([128, N], BF, tag="es")
                    for c in range(2):
                        c0, c1 = 512 * c, 512 * c + 512
                        pst = stp.tile([128, 512], F32, tag="pst")
                        # accumulation group: the full-width main k^T q matmul
                        # runs first with start=True (clears the bank and sets
                        # has_written everywhere), then banded identity-matmuls
                        # and saturation rank-1 matmuls accumulate into their
                        # sub-ranges; the last one closes the group.
                        subs = []
                        for ib in range(ib_lo, ib_hi):
                            if c0 <= 128 * ib < c1:
                                subs.append(("band", ib))
                        if lo_end > c0:
                            subs.append(("lo", (c0, min(lo_end, c1))))
                        if hi_st < c1:
                            subs.append(("hi", (max(hi_st, c0), c1)))
                        nc.tensor.matmul(pst, kTh[hsl, rsl], qTh[hsl, c0:c1],
                                         start=True, stop=False)
                        for j, (kind, arg) in enumerate(subs):
                            sp = (j == len(subs) - 1)
                            if kind == "band":
                                ib = arg
                                q = rb - ib + 4
                                a = 128 * ib
                                nc.tensor.matmul(
                                    pst[:, a - c0:a - c0 + 128], iden_sb,
                                    g[:, q, a:a + 128], start=False, stop=sp)
                            else:
                                a, bnd = arg
                                r = 0 if kind == "lo" else 1
                                nc.tensor.matmul(
                                    pst[:, a - c0:bnd - c0], ones_sb,
                                    gsat[r][:, a:bnd], start=False, stop=sp)
                        nc.scalar.activation(es[:, c0:c1], pst,
                                             mybir.ActivationFunctionType.Exp)
                        nc.tensor.matmul(oacc[c], va_sb[rb][:, 65 * h:65 * h + 65],
                                         es[:, c0:c1],
                                         start=(rb == 0), stop=(rb == NB - 1))

                # --- normalize: oT = oacc[0:64] * (1/Z) ---
                # broadcast Z across partitions via ones x z matmul, then
                # reciprocal (PSUM -> SBUF) and elementwise multiply.
                for c in range(2):
                    csl = slice(512 * c, 512 * c + 512)
                    zb = small.tile([1, 512], BF, tag="zb")
                    nc.vector.tensor_copy(zb, oacc[c][64:65, :])
                    bz = gsp.tile([64, 512], F32, tag="ps", name="bz")
                    nc.tensor.matmul(bz, ones_sb[:, :64], zb,
                                     start=True, stop=True)
                    rb64 = small.tile([64, 512], F32, tag="rb64")
                    nc.vector.reciprocal(rb64, bz)
                    nc.vector.tensor_mul(oT_sb[pair][hsl, csl],
                                         oacc[c][0:64, :], rb64)

        # ---------- output projection ----------
        with tc.tile_pool(name="oproj_psum", bufs=4, space="PSUM") as opp, \
                tc.tile_pool(name="osb", bufs=4) as osb:
            for m in range(4):
                msl = slice(128 * m, 128 * m + 128)
                for c in range(2):
                    csl = slice(512 * c, 512 * c + 512)
                    po = opp.tile([128, 512], F32, tag="po")
                    for k in range(4):
                        nc.tensor.matmul(po, wo_sb[k][:, msl], oT_sb[k][:, csl],
                                         start=(k == 0), stop=(k == 3))
                    ot = osb.tile([128, 512], F32, tag="otf")
                    nc.scalar.add(out=ot, in_=po, add=bo_sb[:, m:m + 1])
                    nc.sync.dma_start(out=out[msl, csl], in_=ot)
    nc.compile()
    return nc


_NC_CACHE = {}


def _get_nc():
    if "nc" not in _NC_CACHE:
        _NC_CACHE["nc"] = build_bass()
    return _NC_CACHE["nc"]


def _host_prep(x, Wq, Wkv, Wo, bo, E):
    u = np.clip(639 - (np.arange(WGX) - 1), -512, 512) + 512
    etxr = E[u].T.astype(bf16)                                   # (64, WGX) padded
    etxr = np.concatenate([etxr, etxr], axis=0)                  # duplicate rows
    et2 = E[[0, 2 * MAX_POS]].T.astype(bf16)                     # (64, 2)
    et2 = np.concatenate([et2, et2], axis=0)
    ii = np.arange(N)
    idx = (10 * ii + 1 + ii // 128).astype(np.int16)
    idxs = np.zeros((16, N // 16), np.int16)
    idxs[ii % 16, ii // 16] = idx
    idxs = np.tile(idxs, (8, 1))                                 # (128, 64)
    common = {
        "wq": (Wq * SCALE).astype(bf16),
        "wk": Wkv[:, :DIM].astype(bf16),
        "wv": Wkv[:, DIM:].astype(bf16),
        "wo": Wo.astype(bf16),
        "bo": np.ascontiguousarray(bo.reshape(4, 128).T.astype(np.float32)),
        "etxr": np.ascontiguousarray(etxr),
        "et2": np.ascontiguousarray(et2),
        "iden": np.eye(128, dtype=bf16),
        "idxs": idxs,
    }
    in_maps = []
    for b in range(B):
        m = dict(common)
        m["xT"] = np.ascontiguousarray(x[b].T.astype(bf16))
        in_maps.append(m)
    return in_maps


def kernel(x, Wq, Wkv, Wo, bo, E):
    x, Wq, Wkv, Wo, bo, E = (np.asarray(a) for a in (x, Wq, Wkv, Wo, bo, E))
    nc = _get_nc()
    in_maps = _host_prep(x, Wq, Wkv, Wo, bo, E)
    res = run_bass_kernel_spmd(nc, in_maps, core_ids=list(range(B)))
    out = np.stack([np.asarray(res.results[b]["out"], dtype=np.float32).T
                    for b in range(B)])
    return out


if __name__ == "__main__":
    rng = np.random.default_rng(0)
    inputs = {
        "x": rng.standard_normal((B, N, DIM), dtype=np.float32),
        "Wq": rng.standard_normal((DIM, H * DH), dtype=np.float32) * DIM ** -0.5,
        "Wkv": rng.standard_normal((DIM, 2 * H * DH), dtype=np.float32) * DIM ** -0.5,
        "Wo": rng.standard_normal((H * DH, DIM), dtype=np.float32) * (H * DH) ** -0.5,
        "bo": np.zeros((DIM,), np.float32),
        "E": rng.standard_normal((2 * MAX_POS + 1, DH), dtype=np.float32),
    }
    o = kernel(**inputs)
    print("kernel ran, out shape", o.shape, "sample", o[0, 0, :4])
